# revision 41
# baseline (speedup 1.0000x reference)
"""3-layer GraphSAGE (mean aggregation) on 8 TRN2 NeuronCores — v3.

Strategy (graph/data parallel):
  - Nodes sharded by contiguous range across 8 cores; each core owns the
    segment-sum targets (edge destinations) in its range.
  - Message tables are fp8(e4m3), split into NCHUNK per-chunk tensors in a
    chunk-permuted row order. Each chunk's table is AllGather'ed separately
    as soon as the tiles feeding it finish, so next-layer gathers (grouped
    per chunk) start without waiting for the whole layer. The last chunk is
    a single tile so the final AG is tiny. Chunk blocks are < 32768 rows so
    int16 gather indices address them directly.
  - Per-edge messages fetched with dma_gather (256B descriptors) round-
    robined over the 4 SWDGE queues. SWDGE descriptor generation (~8ns/idx
    per queue, 4 queues in parallel) is the critical resource; the token
    stream is padded to 128/tile-chunk granularity only.
  - Segment-sum on the TensorEngine with a one-hot S matrix (pure 0/1, fp8)
    built ON-CHIP by a DVE is_equal against a baked iota — no S stream from
    HBM. Inverse-degree scaling applied after aggregation (per-partition
    tensor_scalar).
  - Layers 1/2 aggregate y = relu(h) @ Wl instead of h (projection before
    gather): the aggregated sum is directly the Wl-term of the output, so
    the mean->transpose->matmul path disappears. Layer 0 keeps the classic
    mean@Wl path (its table is x, available at start).
"""

import sys

sys.path.insert(0, "/opt/trn_rl_repo")

import numpy as np
import ml_dtypes

import concourse.bass as bass
import concourse.bacc as bacc
import concourse.mybir as mybir
import concourse.tile as tile
from concourse.bass_utils import run_bass_kernel_spmd
from concourse.masks import make_identity

BF16 = ml_dtypes.bfloat16
F8 = ml_dtypes.float8_e4m3fn
P = 128


class Cfg:
    def __init__(self, n_nodes=50000, dim=256, n_layers=3, n_cores=8,
                 batch_tiles=2, max_gather_chunks=6, msg_bufs=42):
        assert n_nodes % n_cores == 0
        self.N = n_nodes
        self.D = dim
        self.L = n_layers
        self.C = n_cores
        self.NS = n_nodes // n_cores          # nodes per core
        self.T = (self.NS + P - 1) // P       # dst tiles per core
        self.NSP = self.T * P                 # padded nodes per core
        self.BATCH = batch_tiles              # dst tiles per PSUM batch
        self.MAXGC = max_gather_chunks        # chunks (of 128 tokens) per dma_gather
        self.MSGB = msg_bufs
        self.KC = dim // P                    # feature k-chunks (2 for D=256)
        # AG chunks in tiles: few large chunks (each AllGather costs ~25-50us
        # of serialized collective handshake), small tail chunk so the last
        # AllGather lands soon after the layer's last batch.
        T = self.T
        if T >= 30:
            cts = [22, 22, T - 44]
        elif T >= 15:
            cts = [T // 2, T - T // 2]
        else:
            cts = [T]
        self.CTS = cts
        self.NCHUNK = len(cts)
        tile_starts = np.concatenate([[0], np.cumsum(cts)])     # in tiles
        self.Sq = np.minimum(tile_starts * P, self.NS)          # core-local row starts
        self.cks = np.diff(self.Sq)                             # rows per chunk per core
        assert self.cks.sum() == self.NS
        self.Bq = np.concatenate([[0], np.cumsum(self.cks * n_cores)])  # table row blocks
        assert int((self.cks * n_cores).max()) < 32768          # int16 idx per block
        self.chunk_of_tile = []
        for t in range(T):
            q = int(np.searchsorted(tile_starts, t, side="right") - 1)
            self.chunk_of_tile.append(min(q, self.NCHUNK - 1))

    def prow_of(self, node):
        """Permuted table row for global node ids (vectorized)."""
        node = np.asarray(node, np.int64)
        c, r = node // self.NS, node % self.NS
        q = np.searchsorted(self.Sq, r, side="right") - 1
        q = np.minimum(q, self.NCHUNK - 1)
        return self.Bq[q] + c * self.cks[q] + (r - self.Sq[q])


class Schedule:
    """Compile-time token-stream schedule, shared by all cores."""

    def __init__(self, cfg, counts):
        # counts: [C, T, NCHUNK] real-edge counts per (core, dst tile, src chunk)
        self.cfg = cfg
        NCH = cfg.NCHUNK
        mx = counts.max(axis=0)                         # [T, NCH]
        self.M = ((mx + P - 1) // P) * P                # padded slots per (t, q)
        empty = self.M.sum(axis=1) == 0                 # tiles with no edges at all
        self.M[empty, 0] = P                            # still need the dense path
        self.group_off = np.zeros((cfg.T, NCH), np.int64)
        self.segments = []  # (q, tok_off, ntok, [(t, k0_chunks, nk_chunks)], batch_idx)
        tok = 0
        t0 = 0
        bi = 0
        while t0 < cfg.T:
            tiles = range(t0, min(t0 + cfg.BATCH, cfg.T))
            for q in range(NCH):
                seg_tiles = []
                seg_off = tok
                for t in tiles:
                    m = int(self.M[t, q])
                    if m == 0:
                        continue
                    self.group_off[t, q] = tok
                    seg_tiles.append((t, (tok - seg_off) // P, m // P))
                    tok += m
                if seg_tiles:
                    self.segments.append((q, seg_off, tok - seg_off, seg_tiles, bi))
            t0 += cfg.BATCH
            bi += 1
        self.TOT = tok                                   # total tokens per layer
        self.NBATCH = bi
        assert self.TOT % P == 0


def _preprocess(cfg, edge_src, edge_dst):
    """Sort/pad edges per core; build idx, dst-id and inv-degree streams."""
    src = np.asarray(edge_src).astype(np.int64)
    dst = np.asarray(edge_dst).astype(np.int64)
    deg = np.bincount(dst, minlength=cfg.N)
    denom = np.maximum(deg, 1).astype(np.float32)
    inv = (1.0 / denom).astype(np.float32)

    prow = cfg.prow_of(src)
    q_e = np.searchsorted(cfg.Bq, prow, side="right") - 1
    idxv_e = (prow - cfg.Bq[q_e]).astype(np.int16)

    core = dst // cfg.NS
    percore = []
    counts = np.zeros((cfg.C, cfg.T, cfg.NCHUNK), np.int64)
    for c in range(cfg.C):
        m = core == c
        iv = idxv_e[m]
        dl = dst[m] - c * cfg.NS
        t = dl // P
        h = q_e[m]
        order = np.lexsort((h, t))
        iv, dl, t, h = iv[order], dl[order], t[order], h[order]
        np.add.at(counts[c], (t, h), 1)
        percore.append((iv, dl, t, h))

    sched = Schedule(cfg, counts)
    TOT = sched.TOT
    TOTC = TOT // P

    idx_streams = []
    dst_streams = []
    inv_tabs = []
    for c in range(cfg.C):
        iv, dl, t, h = percore[c]
        gid = t * cfg.NCHUNK + h
        changes = np.empty(len(gid), bool)
        if len(gid):
            changes[0] = True
            changes[1:] = gid[1:] != gid[:-1]
        starts = np.flatnonzero(changes)
        rank = np.arange(len(gid)) - np.repeat(starts, np.diff(np.append(starts, len(gid))))
        tok = sched.group_off[t, h] + rank              # token slot per edge

        idx_val = np.zeros(TOT, np.int16)               # pad -> row 0 (zeroed by S)
        idx_val[tok] = iv

        dst_col = np.full((P, TOTC), 999.0, BF16)       # pad -> no iota match
        dst_col[tok % P, tok // P] = (dl % P).astype(BF16)

        # wrapped idx layout per gather segment, then replicated to 128 partitions
        wrapped_cols = []
        for (_q, off, ntok, _tiles, _bi) in sched.segments:
            seg = idx_val[off:off + ntok]
            wrapped_cols.append(seg.reshape(ntok // 16, 16).T)
        wrapped = np.concatenate(wrapped_cols, axis=1)   # [16, TOT//16]
        idx_streams.append(np.ascontiguousarray(np.tile(wrapped, (8, 1))))
        dst_streams.append(np.ascontiguousarray(dst_col))

        it = np.ones((cfg.T * P,), np.float32)
        dn = np.ones((cfg.T * P,), np.float32)
        rows = np.arange(cfg.NSP)
        valid = rows < cfg.NS
        it[valid] = inv[c * cfg.NS + rows[valid]]
        dn[valid] = denom[c * cfg.NS + rows[valid]]
        inv_tabs.append((np.ascontiguousarray(it.reshape(cfg.T, P).T),
                         np.ascontiguousarray(dn.reshape(cfg.T, P).T),
                         np.ascontiguousarray(dn.reshape(1, cfg.T * P)).astype(BF16)))

    return sched, idx_streams, dst_streams, inv_tabs


def _gather_pieces(sched, cfg):
    """Split each segment into BALANCED pieces of <= MAXGC chunks.

    Uniform piece sizes keep the 4-queue round-robin in lock-step at full
    ucode overlap (uneven sizes starve queues)."""
    out = []
    for (_q, _off, ntok, _tiles, _bi) in sched.segments:
        nk = ntok // P
        n = (nk + cfg.MAXGC - 1) // cfg.MAXGC
        base, rem = divmod(nk, n)
        pieces = []
        k = 0
        for i in range(n):
            pk = base + (1 if i < rem else 0)
            pieces.append((k, pk))
            k += pk
        out.append(pieces)
    return out


def _build(cfg, sched):
    nc = bacc.Bacc("TRN2", target_bir_lowering=False, debug=False,
                   num_devices=cfg.C, num_swdge_queues=4)
    dt = mybir.dt
    N, D, NS, NSP, T, KC = cfg.N, cfg.D, cfg.NS, cfg.NSP, cfg.T, cfg.KC
    NCH = cfg.NCHUNK
    TOT = sched.TOT
    TOTC = TOT // P

    # ---- I/O ----
    x_tbl = nc.dram_tensor("x_tbl", [N, D], dt.float8e4, kind="ExternalInput")
    xT_loc = nc.dram_tensor("xT_loc", [P, KC, NSP], dt.bfloat16, kind="ExternalInput")
    idx16 = nc.dram_tensor("idx16", [P, TOT // 16], dt.int16, kind="ExternalInput")
    dst16 = nc.dram_tensor("dst16", [P, TOTC], dt.bfloat16, kind="ExternalInput")
    iota_in = nc.dram_tensor("iota_in", [P, P], dt.bfloat16, kind="ExternalInput")
    inv_in = nc.dram_tensor("inv_in", [P, T], dt.float32, kind="ExternalInput")
    degr_in = nc.dram_tensor("degr_in", [1, NSP], dt.bfloat16, kind="ExternalInput")
    degc_in = nc.dram_tensor("degc_in", [P, T], dt.float32, kind="ExternalInput")
    w_in = {}
    for l in range(cfg.L):
        w_in[("Wl", l)] = nc.dram_tensor(f"Wl{l}b", [KC, P, D], dt.bfloat16, kind="ExternalInput")
        w_in[("Wr", l)] = nc.dram_tensor(f"Wr{l}b", [KC, P, D], dt.bfloat16, kind="ExternalInput")
        w_in[("b", l)] = nc.dram_tensor(f"b{l}b", [1, D], dt.bfloat16, kind="ExternalInput")
    out_ext = nc.dram_tensor("out", [NS, cfg.L, D], dt.float32, kind="ExternalOutput")

    # ---- internal DRAM ----
    ag_y = [[nc.dram_tensor(f"ag_y{l}_{q}", [int(cfg.cks[q]), D], dt.float8e4)
             for q in range(NCH)] for l in range(cfg.L - 1)]
    y_full = [[nc.dram_tensor(f"y_full{l}_{q}", [int(cfg.cks[q]) * cfg.C, D],
                              dt.float8e4, addr_space="Shared")
               for q in range(NCH)] for l in range(cfg.L - 1)]

    pieces_per_seg = _gather_pieces(sched, cfg)
    # AG chunk q covers tiles [sum(CTS[:q]), sum(CTS[:q+1])): fire after the
    # batch containing its last tile.
    tile_starts = np.concatenate([[0], np.cumsum(cfg.CTS)])
    last_batch_of_chunk = [min(int(tile_starts[q + 1]) - 1, T - 1) // cfg.BATCH
                           for q in range(NCH)]
    last_q = {t: max(q for q in range(NCH) if sched.M[t, q] > 0) for t in range(T)}

    with tile.TileContext(nc) as tc:
        with (
            tc.tile_pool(name="const", bufs=1) as constp,
            tc.tile_pool(name="sbuf", bufs=4) as sb,
            tc.tile_pool(name="sseg", bufs=4) as ssegp,
            tc.tile_pool(name="msgp", bufs=cfg.MSGB) as msgp,
            tc.tile_pool(name="psum", bufs=2, space="PSUM") as ps,
            tc.tile_pool(name="psumT", bufs=2, space="PSUM") as psT,
            tc.tile_pool(name="aggp", bufs=2 * cfg.BATCH, space="PSUM") as aggps,
        ):
            # persistent constants
            ident = constp.tile([P, P], dt.bfloat16, tag="ident")
            make_identity(nc, ident[:, :])
            ones_row = constp.tile([1, P], dt.bfloat16, tag="ones")
            nc.gpsimd.memset(ones_row[:, :], 1.0)
            iota_sb = constp.tile([P, P], dt.bfloat16, tag="iota")
            nc.sync.dma_start(out=iota_sb[:, :], in_=iota_in[:, :])
            idx_sb = constp.tile([P, TOT // 16], dt.int16, tag="idx")
            # load in pieces at segment boundaries so the first gathers
            # don't wait for the whole stream
            seg_bounds = [s[1] // 16 for s in sched.segments[::8]] + [TOT // 16]
            for a, b in zip(seg_bounds, seg_bounds[1:]):
                if b > a:
                    nc.sync.dma_start(out=idx_sb[:, a:b], in_=idx16[:, a:b])
            dst_sb = constp.tile([P, TOTC], dt.bfloat16, tag="dst")
            nc.sync.dma_start(out=dst_sb[:, :], in_=dst16[:, :])
            inv_sb = constp.tile([P, T], dt.float32, tag="inv")
            nc.sync.dma_start(out=inv_sb[:, :], in_=inv_in[:, :])
            degr_sb = constp.tile([1, NSP], dt.bfloat16, tag="degr")
            nc.sync.dma_start(out=degr_sb[:, :], in_=degr_in[:, :])
            degc_sb = constp.tile([P, T], dt.float32, tag="degc")
            nc.sync.dma_start(out=degc_sb[:, :], in_=degc_in[:, :])
            w_sb = {}
            for l in range(cfg.L):
                for nm in ("Wl", "Wr"):
                    w = constp.tile([P, KC, D], dt.bfloat16, tag=f"{nm}{l}")
                    for k in range(KC):
                        nc.sync.dma_start(out=w[:, k, :], in_=w_in[(nm, l)][k, :, :])
                    w_sb[(nm, l)] = w
                bt = constp.tile([1, D], dt.bfloat16, tag=f"b{l}")
                nc.sync.dma_start(out=bt[:, :], in_=w_in[("b", l)][:, :])
                w_sb[("b", l)] = bt
            # hprev transposed, ping-pong
            hT = [constp.tile([P, KC, NSP], dt.bfloat16, tag=f"hT{i}",
                              name=f"hT{i}") for i in range(2)]
            for k in range(KC):
                nc.sync.dma_start(out=hT[0][:, k, :], in_=xT_loc[:, k, :])

            gq = [0]
            for l in range(cfg.L):
                if l == 0:
                    tables = [x_tbl[int(cfg.Bq[q]):int(cfg.Bq[q + 1]), :]
                              for q in range(NCH)]
                else:
                    tables = [y_full[l - 1][q][:, :] for q in range(NCH)]
                hT_cur = hT[l % 2]
                hT_nxt = hT[(l + 1) % 2]

                def phase2(pend, l=l, hT_cur=hT_cur, hT_nxt=hT_nxt):
                    if pend is None:
                        return
                    pbi, pitems = pend
                    for (t, rows, buf) in pitems:
                        if l == 0:
                            mean_sb = buf
                            meanT = sb.tile([P, KC, P], dt.bfloat16, tag="meanT")
                            for k in range(KC):
                                tp = psT.tile([P, P], dt.bfloat16, tag="tp")
                                nc.tensor.transpose(
                                    out=tp[:, :],
                                    in_=mean_sb[:, k * P:(k + 1) * P],
                                    identity=ident[:, :])
                                nc.vector.tensor_copy(out=meanT[:, k, :],
                                                      in_=tp[:, :])
                            hp = ps.tile([P, D], dt.float32, tag="hp")
                            nc.tensor.matmul(hp[:, :], lhsT=ones_row[:, :],
                                             rhs=w_sb[("b", l)][:, :],
                                             start=True, stop=False)
                            for k in range(KC):
                                nc.tensor.matmul(hp[:, :], lhsT=meanT[:, k, :],
                                                 rhs=w_sb[("Wl", l)][:, k, :],
                                                 start=False, stop=False)
                            for k in range(KC):
                                nc.tensor.matmul(hp[:, :],
                                                 lhsT=hT_cur[:, k, t * P:(t + 1) * P],
                                                 rhs=w_sb[("Wr", l)][:, k, :],
                                                 start=False, stop=k == KC - 1)
                            hout = sb.tile([P, D], dt.float32, tag="hout")
                            nc.vector.tensor_copy(out=hout[:, :], in_=hp[:, :])
                            nc.scalar.dma_start(
                                out=out_ext[t * P:t * P + rows, l, :],
                                in_=hout[0:rows, :])
                            hrelu = sb.tile([P, D], dt.bfloat16, tag="hrelu")
                            nc.scalar.activation(
                                out=hrelu[:, :], in_=hp[:, :],
                                func=mybir.ActivationFunctionType.Relu,
                                scale=degc_sb[:, t:t + 1])
                        else:
                            hrelu = buf
                        if l < cfg.L - 1:
                            for k in range(KC):
                                tq = psT.tile([P, P], dt.bfloat16, tag="tp")
                                nc.tensor.transpose(
                                    out=tq[:, :],
                                    in_=hrelu[:, k * P:(k + 1) * P],
                                    identity=ident[:, :])
                                nc.vector.tensor_copy(
                                    out=hT_nxt[:, k, t * P:(t + 1) * P],
                                    in_=tq[:, :])
                            yp = ps.tile([P, D], dt.float32, tag="hp")
                            for k in range(KC):
                                nc.tensor.matmul(yp[:, :],
                                                 lhsT=hT_nxt[:, k, t * P:(t + 1) * P],
                                                 rhs=w_sb[("Wl", l + 1)][:, k, :],
                                                 start=k == 0, stop=k == KC - 1)
                            y_sb = sb.tile([P, D], dt.float8e4, tag="ysb")
                            nc.scalar.activation(
                                out=y_sb[:, :], in_=yp[:, :],
                                func=mybir.ActivationFunctionType.Copy,
                                scale=inv_sb[:, t:t + 1])
                            qt = cfg.chunk_of_tile[t]
                            r0 = t * P - int(cfg.Sq[qt])
                            nc.sync.dma_start(
                                out=ag_y[l][qt][r0:r0 + rows, :],
                                in_=y_sb[0:rows, :])
                    # fire AllGather chunks whose tiles are now all written
                    if l < cfg.L - 1:
                        for qq in range(NCH):
                            if last_batch_of_chunk[qq] == pbi:
                                nc.gpsimd.collective_compute(
                                    "AllGather",
                                    mybir.AluOpType.bypass,
                                    replica_groups=[list(range(cfg.C))],
                                    ins=[ag_y[l][qq][:, :]],
                                    outs=[y_full[l][qq][:, :]],
                                )

                pending = None
                agg_of = {}
                first_mm = {}
                nseg = len(sched.segments)
                for si, (q, tok_off, ntok, seg_tiles, bi) in enumerate(sched.segments):
                    nch_seg = ntok // P
                    for (t, k0, nk) in seg_tiles:
                        if t not in agg_of:
                            agg_of[t] = aggps.tile([P, D], dt.float32, tag="agg",
                                                   name=f"agg_l{l}_t{t}")[:, :]
                            first_mm[t] = True
                            if l > 0:
                                # deg*(b + h@Wr) accumulates with the messages;
                                # one inv scale at the end recovers h.
                                nc.tensor.matmul(
                                    agg_of[t],
                                    lhsT=degr_sb[0:1, t * P:(t + 1) * P],
                                    rhs=w_sb[("b", l)][:, :],
                                    start=True, stop=False)
                                for k in range(KC):
                                    nc.tensor.matmul(
                                        agg_of[t],
                                        lhsT=hT_cur[:, k, t * P:(t + 1) * P],
                                        rhs=w_sb[("Wr", l)][:, k, :],
                                        start=False, stop=False)
                                first_mm[t] = False
                    # gather pieces (round-robin the 4 SWDGE queues)
                    msg_tiles = []
                    for (pk0, pnk) in pieces_per_seg[si]:
                        mt = msgp.tile([P, cfg.MAXGC, D], dt.float8e4, tag="msg")
                        ntk = pnk * P
                        c0 = (tok_off + pk0 * P) // 16
                        nc.gpsimd.dma_gather(
                            mt[:, 0:pnk, :],
                            tables[q],
                            idx_sb[:, c0:c0 + ntk // 16],
                            ntk, ntk, D,
                            single_packet=False,
                            queue_num=gq[0] % 4,
                        )
                        gq[0] += 1
                        msg_tiles.append(mt)
                    # S columns for this segment, built on-chip:
                    # S[tok, ch*128 + d] = (dst_col[tok, ch] == iota[d])
                    s_sb = ssegp.tile([P, nch_seg, P], dt.float8e4, tag="sseg")
                    cc0 = tok_off // P
                    nc.vector.tensor_tensor(
                        out=s_sb[:, :, :],
                        in0=dst_sb[:, cc0:cc0 + nch_seg].unsqueeze(2)
                            .broadcast_to((P, nch_seg, P)),
                        in1=iota_sb[:, :].unsqueeze(1)
                            .broadcast_to((P, nch_seg, P)),
                        op=mybir.AluOpType.is_equal,
                    )
                    # aggregation matmuls (fp8)
                    cmap = {}
                    for pi, (pk0, pnk) in enumerate(pieces_per_seg[si]):
                        for j2 in range(pnk):
                            cmap[pk0 + j2] = (pi, j2)
                    for (t, k0, nk) in seg_tiles:
                        for j in range(nk):
                            ch = k0 + j
                            pi, loc = cmap[ch]
                            nc.tensor.matmul(
                                agg_of[t],
                                lhsT=s_sb[:, ch, :],
                                rhs=msg_tiles[pi][:, loc, :],
                                start=first_mm[t],
                                stop=(q == last_q[t]) and j + 1 >= nk,
                            )
                            first_mm[t] = False
                    # after the last segment of the batch: finish its tiles
                    batch_done = (si + 1 == nseg) or sched.segments[si + 1][4] != bi
                    if not batch_done:
                        continue
                    # phase2 of the PREVIOUS batch first (its cross-engine
                    # inputs are ready by now, so its PE ops don't stall the
                    # in-order matmul queue)
                    phase2(pending)
                    items = []
                    for t in sorted(agg_of):
                        aggt = agg_of[t]
                        rows = min(P, NS - t * P)
                        if l == 0:
                            # mean (bf16); dense path happens in phase2
                            mean_sb = sb.tile([P, D], dt.bfloat16, tag="mean")
                            nc.vector.tensor_scalar(
                                out=mean_sb[:, :], in0=aggt,
                                scalar1=inv_sb[:, t:t + 1], scalar2=None,
                                op0=mybir.AluOpType.mult)
                            items.append((t, rows, mean_sb))
                        else:
                            # h = inv * (sum msgs + deg*(b + h@Wr))
                            hout = sb.tile([P, D], dt.float32, tag="hout")
                            nc.vector.tensor_scalar(
                                out=hout[:, :], in0=aggt,
                                scalar1=inv_sb[:, t:t + 1], scalar2=None,
                                op0=mybir.AluOpType.mult)
                            nc.scalar.dma_start(
                                out=out_ext[t * P:t * P + rows, l, :],
                                in_=hout[0:rows, :])
                            hrelu = None
                            if l < cfg.L - 1:
                                hrelu = sb.tile([P, D], dt.bfloat16, tag="hrelu")
                                # scale applies before Relu; deg>0 so
                                # Relu(deg*h) == deg*Relu(h)
                                nc.scalar.activation(
                                    out=hrelu[:, :], in_=hout[:, :],
                                    func=mybir.ActivationFunctionType.Relu,
                                    scale=degc_sb[:, t:t + 1])
                            items.append((t, rows, hrelu))
                    if l == 0:
                        phase2((bi, items))
                        pending = None
                    else:
                        pending = (bi, items)
                    agg_of = {}
                    first_mm = {}
                # end of layer: flush the final batch's phase2
                phase2(pending)

    nc.compile()
    return nc


def _prepare_inputs(cfg, inputs):
    """Host-side shard/pack. Returns (sched, per-core input maps)."""
    x = np.asarray(inputs["x"], np.float32)
    sched, idx_streams, dst_streams, inv_tabs = _preprocess(
        cfg, inputs["edge_src"], inputs["edge_dst"])

    x_bf = x.astype(BF16)
    # fp8 table in permuted row order
    perm = cfg.prow_of(np.arange(cfg.N))
    x_f8p = np.empty((cfg.N, cfg.D), F8)
    x_f8p[perm] = x_bf.astype(F8)
    x_f8p = np.ascontiguousarray(x_f8p)

    iota = np.tile(np.arange(P, dtype=np.float32).astype(BF16)[None, :], (P, 1))

    in_maps = []
    for c in range(cfg.C):
        xc = x_bf[c * cfg.NS:(c + 1) * cfg.NS]           # [NS, D]
        xT = np.zeros((cfg.D, cfg.NSP), BF16)
        xT[:, :cfg.NS] = xc.T
        xT = np.ascontiguousarray(
            xT.reshape(cfg.KC, P, cfg.NSP).transpose(1, 0, 2))
        m = {
            "x_tbl": x_f8p,
            "xT_loc": xT,
            "idx16": idx_streams[c],
            "dst16": dst_streams[c],
            "iota_in": np.ascontiguousarray(iota),
            "inv_in": inv_tabs[c][0],
            "degc_in": inv_tabs[c][1],
            "degr_in": inv_tabs[c][2],
        }
        for l in range(cfg.L):
            wl = np.asarray(inputs[f"Wl{l}"], np.float32).astype(BF16)
            wr = np.asarray(inputs[f"Wr{l}"], np.float32).astype(BF16)
            bb = np.asarray(inputs[f"b{l}"], np.float32).astype(BF16)
            m[f"Wl{l}b"] = np.ascontiguousarray(wl.reshape(cfg.KC, P, cfg.D))
            m[f"Wr{l}b"] = np.ascontiguousarray(wr.reshape(cfg.KC, P, cfg.D))
            m[f"b{l}b"] = np.ascontiguousarray(bb.reshape(1, cfg.D))
        in_maps.append(m)
    return sched, in_maps


_CACHE = {}


def run(inputs, cfg=None, trace=False):
    cfg = cfg or Cfg()
    sched, in_maps = _prepare_inputs(cfg, inputs)
    key = (cfg.N, cfg.D, cfg.C, tuple(sched.M.ravel()))
    if key not in _CACHE:
        _CACHE[key] = _build(cfg, sched)
    nc = _CACHE[key]
    res = run_bass_kernel_spmd(nc, in_maps, list(range(cfg.C)), trace=trace)
    out = np.concatenate([res.results[c]["out"] for c in range(cfg.C)], axis=0)
    return out, res


def kernel(**inputs):
    out, _ = run(inputs)
    return out


# revision 42
# speedup vs baseline: 1.0378x; 1.0378x over previous
"""3-layer GraphSAGE (mean aggregation) on 8 TRN2 NeuronCores — v3.

Strategy (graph/data parallel):
  - Nodes sharded by contiguous range across 8 cores; each core owns the
    segment-sum targets (edge destinations) in its range.
  - Message tables are fp8(e4m3), split into NCHUNK per-chunk tensors in a
    chunk-permuted row order. Each chunk's table is AllGather'ed separately
    as soon as the tiles feeding it finish, so next-layer gathers (grouped
    per chunk) start without waiting for the whole layer. The last chunk is
    a single tile so the final AG is tiny. Chunk blocks are < 32768 rows so
    int16 gather indices address them directly.
  - Per-edge messages fetched with dma_gather (256B descriptors) round-
    robined over the 4 SWDGE queues. SWDGE descriptor generation (~8ns/idx
    per queue, 4 queues in parallel) is the critical resource; the token
    stream is padded to 128/tile-chunk granularity only.
  - Segment-sum on the TensorEngine with a one-hot S matrix (pure 0/1, fp8)
    built ON-CHIP by a DVE is_equal against a baked iota — no S stream from
    HBM. Inverse-degree scaling applied after aggregation (per-partition
    tensor_scalar).
  - Layers 1/2 aggregate y = relu(h) @ Wl instead of h (projection before
    gather): the aggregated sum is directly the Wl-term of the output, so
    the mean->transpose->matmul path disappears. Layer 0 keeps the classic
    mean@Wl path (its table is x, available at start).
"""

import sys

sys.path.insert(0, "/opt/trn_rl_repo")

import numpy as np
import ml_dtypes

import concourse.bass as bass
import concourse.bacc as bacc
import concourse.mybir as mybir
import concourse.tile as tile
from concourse.bass_utils import run_bass_kernel_spmd
from concourse.masks import make_identity

BF16 = ml_dtypes.bfloat16
F8 = ml_dtypes.float8_e4m3fn
P = 128


class Cfg:
    def __init__(self, n_nodes=50000, dim=256, n_layers=3, n_cores=8,
                 batch_tiles=2, max_gather_chunks=8, msg_bufs=32):
        assert n_nodes % n_cores == 0
        self.N = n_nodes
        self.D = dim
        self.L = n_layers
        self.C = n_cores
        self.NS = n_nodes // n_cores          # nodes per core
        self.T = (self.NS + P - 1) // P       # dst tiles per core
        self.NSP = self.T * P                 # padded nodes per core
        self.BATCH = batch_tiles              # dst tiles per PSUM batch
        self.MAXGC = max_gather_chunks        # chunks (of 128 tokens) per dma_gather
        self.MSGB = msg_bufs
        self.KC = dim // P                    # feature k-chunks (2 for D=256)
        # AG chunks in tiles: few large chunks (each AllGather costs ~25-50us
        # of serialized collective handshake), small tail chunk so the last
        # AllGather lands soon after the layer's last batch.
        T = self.T
        if T >= 30:
            cts = [22, 22, T - 44]
        elif T >= 15:
            cts = [T // 2, T - T // 2]
        else:
            cts = [T]
        self.CTS = cts
        self.NCHUNK = len(cts)
        tile_starts = np.concatenate([[0], np.cumsum(cts)])     # in tiles
        self.Sq = np.minimum(tile_starts * P, self.NS)          # core-local row starts
        self.cks = np.diff(self.Sq)                             # rows per chunk per core
        assert self.cks.sum() == self.NS
        self.Bq = np.concatenate([[0], np.cumsum(self.cks * n_cores)])  # table row blocks
        assert int((self.cks * n_cores).max()) < 32768          # int16 idx per block
        self.chunk_of_tile = []
        for t in range(T):
            q = int(np.searchsorted(tile_starts, t, side="right") - 1)
            self.chunk_of_tile.append(min(q, self.NCHUNK - 1))

    def prow_of(self, node):
        """Permuted table row for global node ids (vectorized)."""
        node = np.asarray(node, np.int64)
        c, r = node // self.NS, node % self.NS
        q = np.searchsorted(self.Sq, r, side="right") - 1
        q = np.minimum(q, self.NCHUNK - 1)
        return self.Bq[q] + c * self.cks[q] + (r - self.Sq[q])


class Schedule:
    """Compile-time token-stream schedule, shared by all cores."""

    def __init__(self, cfg, counts):
        # counts: [C, T, NCHUNK] real-edge counts per (core, dst tile, src chunk)
        self.cfg = cfg
        NCH = cfg.NCHUNK
        mx = counts.max(axis=0)                         # [T, NCH]
        self.M = ((mx + P - 1) // P) * P                # padded slots per (t, q)
        empty = self.M.sum(axis=1) == 0                 # tiles with no edges at all
        self.M[empty, 0] = P                            # still need the dense path
        self.group_off = np.zeros((cfg.T, NCH), np.int64)
        self.segments = []  # (q, tok_off, ntok, [(t, k0_chunks, nk_chunks)], batch_idx)
        tok = 0
        t0 = 0
        bi = 0
        while t0 < cfg.T:
            tiles = range(t0, min(t0 + cfg.BATCH, cfg.T))
            for q in range(NCH):
                seg_tiles = []
                seg_off = tok
                for t in tiles:
                    m = int(self.M[t, q])
                    if m == 0:
                        continue
                    self.group_off[t, q] = tok
                    seg_tiles.append((t, (tok - seg_off) // P, m // P))
                    tok += m
                if seg_tiles:
                    self.segments.append((q, seg_off, tok - seg_off, seg_tiles, bi))
            t0 += cfg.BATCH
            bi += 1
        self.TOT = tok                                   # total tokens per layer
        self.NBATCH = bi
        assert self.TOT % P == 0


def _preprocess(cfg, edge_src, edge_dst):
    """Sort/pad edges per core; build idx, dst-id and inv-degree streams."""
    src = np.asarray(edge_src).astype(np.int64)
    dst = np.asarray(edge_dst).astype(np.int64)
    deg = np.bincount(dst, minlength=cfg.N)
    denom = np.maximum(deg, 1).astype(np.float32)
    inv = (1.0 / denom).astype(np.float32)

    prow = cfg.prow_of(src)
    q_e = np.searchsorted(cfg.Bq, prow, side="right") - 1
    idxv_e = (prow - cfg.Bq[q_e]).astype(np.int16)

    core = dst // cfg.NS
    percore = []
    counts = np.zeros((cfg.C, cfg.T, cfg.NCHUNK), np.int64)
    for c in range(cfg.C):
        m = core == c
        iv = idxv_e[m]
        dl = dst[m] - c * cfg.NS
        t = dl // P
        h = q_e[m]
        order = np.lexsort((h, t))
        iv, dl, t, h = iv[order], dl[order], t[order], h[order]
        np.add.at(counts[c], (t, h), 1)
        percore.append((iv, dl, t, h))

    sched = Schedule(cfg, counts)
    TOT = sched.TOT
    TOTC = TOT // P

    idx_streams = []
    dst_streams = []
    inv_tabs = []
    for c in range(cfg.C):
        iv, dl, t, h = percore[c]
        gid = t * cfg.NCHUNK + h
        changes = np.empty(len(gid), bool)
        if len(gid):
            changes[0] = True
            changes[1:] = gid[1:] != gid[:-1]
        starts = np.flatnonzero(changes)
        rank = np.arange(len(gid)) - np.repeat(starts, np.diff(np.append(starts, len(gid))))
        tok = sched.group_off[t, h] + rank              # token slot per edge

        idx_val = np.zeros(TOT, np.int16)               # pad -> row 0 (zeroed by S)
        idx_val[tok] = iv

        dst_col = np.full((P, TOTC), 999.0, BF16)       # pad -> no iota match
        dst_col[tok % P, tok // P] = (dl % P).astype(BF16)

        # wrapped idx layout per gather segment, then replicated to 128 partitions
        wrapped_cols = []
        for (_q, off, ntok, _tiles, _bi) in sched.segments:
            seg = idx_val[off:off + ntok]
            wrapped_cols.append(seg.reshape(ntok // 16, 16).T)
        wrapped = np.concatenate(wrapped_cols, axis=1)   # [16, TOT//16]
        idx_streams.append(np.ascontiguousarray(np.tile(wrapped, (8, 1))))
        dst_streams.append(np.ascontiguousarray(dst_col))

        it = np.ones((cfg.T * P,), np.float32)
        dn = np.ones((cfg.T * P,), np.float32)
        rows = np.arange(cfg.NSP)
        valid = rows < cfg.NS
        it[valid] = inv[c * cfg.NS + rows[valid]]
        dn[valid] = denom[c * cfg.NS + rows[valid]]
        inv_tabs.append((np.ascontiguousarray(it.reshape(cfg.T, P).T),
                         np.ascontiguousarray(dn.reshape(cfg.T, P).T),
                         np.ascontiguousarray(dn.reshape(1, cfg.T * P)).astype(BF16)))

    return sched, idx_streams, dst_streams, inv_tabs


def _gather_pieces(sched, cfg):
    """Split each segment into BALANCED pieces of <= MAXGC chunks.

    Uniform piece sizes keep the 4-queue round-robin in lock-step at full
    ucode overlap (uneven sizes starve queues)."""
    out = []
    for (_q, _off, ntok, _tiles, _bi) in sched.segments:
        nk = ntok // P
        n = (nk + cfg.MAXGC - 1) // cfg.MAXGC
        base, rem = divmod(nk, n)
        pieces = []
        k = 0
        for i in range(n):
            pk = base + (1 if i < rem else 0)
            pieces.append((k, pk))
            k += pk
        out.append(pieces)
    return out


def _build(cfg, sched):
    nc = bacc.Bacc("TRN2", target_bir_lowering=False, debug=False,
                   num_devices=cfg.C, num_swdge_queues=4)
    dt = mybir.dt
    N, D, NS, NSP, T, KC = cfg.N, cfg.D, cfg.NS, cfg.NSP, cfg.T, cfg.KC
    NCH = cfg.NCHUNK
    TOT = sched.TOT
    TOTC = TOT // P

    # ---- I/O ----
    x_tbl = nc.dram_tensor("x_tbl", [N, D], dt.float8e4, kind="ExternalInput")
    xT_loc = nc.dram_tensor("xT_loc", [P, KC, NSP], dt.bfloat16, kind="ExternalInput")
    idx16 = nc.dram_tensor("idx16", [P, TOT // 16], dt.int16, kind="ExternalInput")
    dst16 = nc.dram_tensor("dst16", [P, TOTC], dt.bfloat16, kind="ExternalInput")
    iota_in = nc.dram_tensor("iota_in", [P, P], dt.bfloat16, kind="ExternalInput")
    inv_in = nc.dram_tensor("inv_in", [P, T], dt.float32, kind="ExternalInput")
    degr_in = nc.dram_tensor("degr_in", [1, NSP], dt.bfloat16, kind="ExternalInput")
    degc_in = nc.dram_tensor("degc_in", [P, T], dt.float32, kind="ExternalInput")
    w_in = {}
    for l in range(cfg.L):
        w_in[("Wl", l)] = nc.dram_tensor(f"Wl{l}b", [KC, P, D], dt.bfloat16, kind="ExternalInput")
        w_in[("Wr", l)] = nc.dram_tensor(f"Wr{l}b", [KC, P, D], dt.bfloat16, kind="ExternalInput")
        w_in[("b", l)] = nc.dram_tensor(f"b{l}b", [1, D], dt.bfloat16, kind="ExternalInput")
    out_ext = nc.dram_tensor("out", [NS, cfg.L, D], dt.float32, kind="ExternalOutput")

    # ---- internal DRAM ----
    ag_y = [[nc.dram_tensor(f"ag_y{l}_{q}", [int(cfg.cks[q]), D], dt.float8e4)
             for q in range(NCH)] for l in range(cfg.L - 1)]
    y_full = [[nc.dram_tensor(f"y_full{l}_{q}", [int(cfg.cks[q]) * cfg.C, D],
                              dt.float8e4, addr_space="Shared")
               for q in range(NCH)] for l in range(cfg.L - 1)]

    pieces_per_seg = _gather_pieces(sched, cfg)
    # AG chunk q covers tiles [sum(CTS[:q]), sum(CTS[:q+1])): fire after the
    # batch containing its last tile.
    tile_starts = np.concatenate([[0], np.cumsum(cfg.CTS)])
    last_batch_of_chunk = [min(int(tile_starts[q + 1]) - 1, T - 1) // cfg.BATCH
                           for q in range(NCH)]
    last_q = {t: max(q for q in range(NCH) if sched.M[t, q] > 0) for t in range(T)}

    with tile.TileContext(nc) as tc:
        with (
            tc.tile_pool(name="const", bufs=1) as constp,
            tc.tile_pool(name="sbuf", bufs=4) as sb,
            tc.tile_pool(name="sseg", bufs=4) as ssegp,
            tc.tile_pool(name="msgp", bufs=cfg.MSGB) as msgp,
            tc.tile_pool(name="psum", bufs=2, space="PSUM") as ps,
            tc.tile_pool(name="psumT", bufs=2, space="PSUM") as psT,
            tc.tile_pool(name="aggp", bufs=2 * cfg.BATCH, space="PSUM") as aggps,
        ):
            # persistent constants
            ident = constp.tile([P, P], dt.bfloat16, tag="ident")
            make_identity(nc, ident[:, :])
            ones_row = constp.tile([1, P], dt.bfloat16, tag="ones")
            nc.gpsimd.memset(ones_row[:, :], 1.0)
            iota_sb = constp.tile([P, P], dt.bfloat16, tag="iota")
            nc.sync.dma_start(out=iota_sb[:, :], in_=iota_in[:, :])
            idx_sb = constp.tile([P, TOT // 16], dt.int16, tag="idx")
            # load in pieces at segment boundaries so the first gathers
            # don't wait for the whole stream
            seg_bounds = [s[1] // 16 for s in sched.segments[::8]] + [TOT // 16]
            for a, b in zip(seg_bounds, seg_bounds[1:]):
                if b > a:
                    nc.sync.dma_start(out=idx_sb[:, a:b], in_=idx16[:, a:b])
            dst_sb = constp.tile([P, TOTC], dt.bfloat16, tag="dst")
            nc.sync.dma_start(out=dst_sb[:, :], in_=dst16[:, :])
            inv_sb = constp.tile([P, T], dt.float32, tag="inv")
            nc.sync.dma_start(out=inv_sb[:, :], in_=inv_in[:, :])
            degr_sb = constp.tile([1, NSP], dt.bfloat16, tag="degr")
            nc.sync.dma_start(out=degr_sb[:, :], in_=degr_in[:, :])
            degc_sb = constp.tile([P, T], dt.float32, tag="degc")
            nc.sync.dma_start(out=degc_sb[:, :], in_=degc_in[:, :])
            w_sb = {}
            for l in range(cfg.L):
                for nm in ("Wl", "Wr"):
                    w = constp.tile([P, KC, D], dt.bfloat16, tag=f"{nm}{l}")
                    for k in range(KC):
                        nc.sync.dma_start(out=w[:, k, :], in_=w_in[(nm, l)][k, :, :])
                    w_sb[(nm, l)] = w
                bt = constp.tile([1, D], dt.bfloat16, tag=f"b{l}")
                nc.sync.dma_start(out=bt[:, :], in_=w_in[("b", l)][:, :])
                w_sb[("b", l)] = bt
            # hprev transposed, ping-pong
            hT = [constp.tile([P, KC, NSP], dt.bfloat16, tag=f"hT{i}",
                              name=f"hT{i}") for i in range(2)]
            for k in range(KC):
                nc.sync.dma_start(out=hT[0][:, k, :], in_=xT_loc[:, k, :])

            gq = [0]
            for l in range(cfg.L):
                if l == 0:
                    tables = [x_tbl[int(cfg.Bq[q]):int(cfg.Bq[q + 1]), :]
                              for q in range(NCH)]
                else:
                    tables = [y_full[l - 1][q][:, :] for q in range(NCH)]
                hT_cur = hT[l % 2]
                hT_nxt = hT[(l + 1) % 2]

                def phase2(pend, l=l, hT_cur=hT_cur, hT_nxt=hT_nxt):
                    if pend is None:
                        return
                    pbi, pitems = pend
                    for (t, rows, buf) in pitems:
                        if l == 0:
                            mean_sb = buf
                            meanT = sb.tile([P, KC, P], dt.bfloat16, tag="meanT")
                            for k in range(KC):
                                tp = psT.tile([P, P], dt.bfloat16, tag="tp")
                                nc.tensor.transpose(
                                    out=tp[:, :],
                                    in_=mean_sb[:, k * P:(k + 1) * P],
                                    identity=ident[:, :])
                                nc.vector.tensor_copy(out=meanT[:, k, :],
                                                      in_=tp[:, :])
                            hp = ps.tile([P, D], dt.float32, tag="hp")
                            nc.tensor.matmul(hp[:, :], lhsT=ones_row[:, :],
                                             rhs=w_sb[("b", l)][:, :],
                                             start=True, stop=False)
                            for k in range(KC):
                                nc.tensor.matmul(hp[:, :], lhsT=meanT[:, k, :],
                                                 rhs=w_sb[("Wl", l)][:, k, :],
                                                 start=False, stop=False)
                            for k in range(KC):
                                nc.tensor.matmul(hp[:, :],
                                                 lhsT=hT_cur[:, k, t * P:(t + 1) * P],
                                                 rhs=w_sb[("Wr", l)][:, k, :],
                                                 start=False, stop=k == KC - 1)
                            hout = sb.tile([P, D], dt.float32, tag="hout")
                            nc.vector.tensor_copy(out=hout[:, :], in_=hp[:, :])
                            nc.scalar.dma_start(
                                out=out_ext[t * P:t * P + rows, l, :],
                                in_=hout[0:rows, :])
                            hrelu = sb.tile([P, D], dt.bfloat16, tag="hrelu")
                            nc.scalar.activation(
                                out=hrelu[:, :], in_=hp[:, :],
                                func=mybir.ActivationFunctionType.Relu,
                                scale=degc_sb[:, t:t + 1])
                        else:
                            hrelu = buf
                        if l < cfg.L - 1:
                            for k in range(KC):
                                tq = psT.tile([P, P], dt.bfloat16, tag="tp")
                                nc.tensor.transpose(
                                    out=tq[:, :],
                                    in_=hrelu[:, k * P:(k + 1) * P],
                                    identity=ident[:, :])
                                nc.vector.tensor_copy(
                                    out=hT_nxt[:, k, t * P:(t + 1) * P],
                                    in_=tq[:, :])
                            yp = ps.tile([P, D], dt.float32, tag="hp")
                            for k in range(KC):
                                nc.tensor.matmul(yp[:, :],
                                                 lhsT=hT_nxt[:, k, t * P:(t + 1) * P],
                                                 rhs=w_sb[("Wl", l + 1)][:, k, :],
                                                 start=k == 0, stop=k == KC - 1)
                            y_sb = sb.tile([P, D], dt.float8e4, tag="ysb")
                            nc.scalar.activation(
                                out=y_sb[:, :], in_=yp[:, :],
                                func=mybir.ActivationFunctionType.Copy,
                                scale=inv_sb[:, t:t + 1])
                            qt = cfg.chunk_of_tile[t]
                            r0 = t * P - int(cfg.Sq[qt])
                            nc.sync.dma_start(
                                out=ag_y[l][qt][r0:r0 + rows, :],
                                in_=y_sb[0:rows, :])
                    # fire AllGather chunks whose tiles are now all written
                    if l < cfg.L - 1:
                        for qq in range(NCH):
                            if last_batch_of_chunk[qq] == pbi:
                                nc.gpsimd.collective_compute(
                                    "AllGather",
                                    mybir.AluOpType.bypass,
                                    replica_groups=[list(range(cfg.C))],
                                    ins=[ag_y[l][qq][:, :]],
                                    outs=[y_full[l][qq][:, :]],
                                )

                pending = None
                agg_of = {}
                first_mm = {}
                nseg = len(sched.segments)
                for si, (q, tok_off, ntok, seg_tiles, bi) in enumerate(sched.segments):
                    nch_seg = ntok // P
                    for (t, k0, nk) in seg_tiles:
                        if t not in agg_of:
                            agg_of[t] = aggps.tile([P, D], dt.float32, tag="agg",
                                                   name=f"agg_l{l}_t{t}")[:, :]
                            first_mm[t] = True
                            if l > 0:
                                # deg*(b + h@Wr) accumulates with the messages;
                                # one inv scale at the end recovers h.
                                nc.tensor.matmul(
                                    agg_of[t],
                                    lhsT=degr_sb[0:1, t * P:(t + 1) * P],
                                    rhs=w_sb[("b", l)][:, :],
                                    start=True, stop=False)
                                for k in range(KC):
                                    nc.tensor.matmul(
                                        agg_of[t],
                                        lhsT=hT_cur[:, k, t * P:(t + 1) * P],
                                        rhs=w_sb[("Wr", l)][:, k, :],
                                        start=False, stop=False)
                                first_mm[t] = False
                    # gather pieces (round-robin the 4 SWDGE queues)
                    msg_tiles = []
                    for (pk0, pnk) in pieces_per_seg[si]:
                        mt = msgp.tile([P, cfg.MAXGC, D], dt.float8e4, tag="msg")
                        ntk = pnk * P
                        c0 = (tok_off + pk0 * P) // 16
                        nc.gpsimd.dma_gather(
                            mt[:, 0:pnk, :],
                            tables[q],
                            idx_sb[:, c0:c0 + ntk // 16],
                            ntk, ntk, D,
                            single_packet=False,
                            queue_num=gq[0] % 4,
                        )
                        gq[0] += 1
                        msg_tiles.append(mt)
                    # S columns for this segment, built on-chip:
                    # S[tok, ch*128 + d] = (dst_col[tok, ch] == iota[d])
                    s_sb = ssegp.tile([P, nch_seg, P], dt.float8e4, tag="sseg")
                    cc0 = tok_off // P
                    nc.vector.tensor_tensor(
                        out=s_sb[:, :, :],
                        in0=dst_sb[:, cc0:cc0 + nch_seg].unsqueeze(2)
                            .broadcast_to((P, nch_seg, P)),
                        in1=iota_sb[:, :].unsqueeze(1)
                            .broadcast_to((P, nch_seg, P)),
                        op=mybir.AluOpType.is_equal,
                    )
                    # aggregation matmuls (fp8)
                    cmap = {}
                    for pi, (pk0, pnk) in enumerate(pieces_per_seg[si]):
                        for j2 in range(pnk):
                            cmap[pk0 + j2] = (pi, j2)
                    for (t, k0, nk) in seg_tiles:
                        for j in range(nk):
                            ch = k0 + j
                            pi, loc = cmap[ch]
                            nc.tensor.matmul(
                                agg_of[t],
                                lhsT=s_sb[:, ch, :],
                                rhs=msg_tiles[pi][:, loc, :],
                                start=first_mm[t],
                                stop=(q == last_q[t]) and j + 1 >= nk,
                            )
                            first_mm[t] = False
                    # after the last segment of the batch: finish its tiles
                    batch_done = (si + 1 == nseg) or sched.segments[si + 1][4] != bi
                    if not batch_done:
                        continue
                    # phase2 of the PREVIOUS batch first (its cross-engine
                    # inputs are ready by now, so its PE ops don't stall the
                    # in-order matmul queue)
                    phase2(pending)
                    items = []
                    for t in sorted(agg_of):
                        aggt = agg_of[t]
                        rows = min(P, NS - t * P)
                        if l == 0:
                            # mean (bf16); dense path happens in phase2
                            mean_sb = sb.tile([P, D], dt.bfloat16, tag="mean")
                            nc.vector.tensor_scalar(
                                out=mean_sb[:, :], in0=aggt,
                                scalar1=inv_sb[:, t:t + 1], scalar2=None,
                                op0=mybir.AluOpType.mult)
                            items.append((t, rows, mean_sb))
                        else:
                            # h = inv * (sum msgs + deg*(b + h@Wr))
                            hout = sb.tile([P, D], dt.float32, tag="hout")
                            nc.vector.tensor_scalar(
                                out=hout[:, :], in0=aggt,
                                scalar1=inv_sb[:, t:t + 1], scalar2=None,
                                op0=mybir.AluOpType.mult)
                            nc.scalar.dma_start(
                                out=out_ext[t * P:t * P + rows, l, :],
                                in_=hout[0:rows, :])
                            hrelu = None
                            if l < cfg.L - 1:
                                hrelu = sb.tile([P, D], dt.bfloat16, tag="hrelu")
                                # scale applies before Relu; deg>0 so
                                # Relu(deg*h) == deg*Relu(h)
                                nc.scalar.activation(
                                    out=hrelu[:, :], in_=hout[:, :],
                                    func=mybir.ActivationFunctionType.Relu,
                                    scale=degc_sb[:, t:t + 1])
                            items.append((t, rows, hrelu))
                    if l == 0:
                        phase2((bi, items))
                        pending = None
                    else:
                        pending = (bi, items)
                    agg_of = {}
                    first_mm = {}
                # end of layer: flush the final batch's phase2
                phase2(pending)

    nc.compile()
    return nc


def _prepare_inputs(cfg, inputs):
    """Host-side shard/pack. Returns (sched, per-core input maps)."""
    x = np.asarray(inputs["x"], np.float32)
    sched, idx_streams, dst_streams, inv_tabs = _preprocess(
        cfg, inputs["edge_src"], inputs["edge_dst"])

    x_bf = x.astype(BF16)
    # fp8 table in permuted row order
    perm = cfg.prow_of(np.arange(cfg.N))
    x_f8p = np.empty((cfg.N, cfg.D), F8)
    x_f8p[perm] = x_bf.astype(F8)
    x_f8p = np.ascontiguousarray(x_f8p)

    iota = np.tile(np.arange(P, dtype=np.float32).astype(BF16)[None, :], (P, 1))

    in_maps = []
    for c in range(cfg.C):
        xc = x_bf[c * cfg.NS:(c + 1) * cfg.NS]           # [NS, D]
        xT = np.zeros((cfg.D, cfg.NSP), BF16)
        xT[:, :cfg.NS] = xc.T
        xT = np.ascontiguousarray(
            xT.reshape(cfg.KC, P, cfg.NSP).transpose(1, 0, 2))
        m = {
            "x_tbl": x_f8p,
            "xT_loc": xT,
            "idx16": idx_streams[c],
            "dst16": dst_streams[c],
            "iota_in": np.ascontiguousarray(iota),
            "inv_in": inv_tabs[c][0],
            "degc_in": inv_tabs[c][1],
            "degr_in": inv_tabs[c][2],
        }
        for l in range(cfg.L):
            wl = np.asarray(inputs[f"Wl{l}"], np.float32).astype(BF16)
            wr = np.asarray(inputs[f"Wr{l}"], np.float32).astype(BF16)
            bb = np.asarray(inputs[f"b{l}"], np.float32).astype(BF16)
            m[f"Wl{l}b"] = np.ascontiguousarray(wl.reshape(cfg.KC, P, cfg.D))
            m[f"Wr{l}b"] = np.ascontiguousarray(wr.reshape(cfg.KC, P, cfg.D))
            m[f"b{l}b"] = np.ascontiguousarray(bb.reshape(1, cfg.D))
        in_maps.append(m)
    return sched, in_maps


_CACHE = {}


def run(inputs, cfg=None, trace=False):
    cfg = cfg or Cfg()
    sched, in_maps = _prepare_inputs(cfg, inputs)
    key = (cfg.N, cfg.D, cfg.C, tuple(sched.M.ravel()))
    if key not in _CACHE:
        _CACHE[key] = _build(cfg, sched)
    nc = _CACHE[key]
    res = run_bass_kernel_spmd(nc, in_maps, list(range(cfg.C)), trace=trace)
    out = np.concatenate([res.results[c]["out"] for c in range(cfg.C)], axis=0)
    return out, res


def kernel(**inputs):
    out, _ = run(inputs)
    return out
